# revision 41
# baseline (speedup 1.0000x reference)
"""Trainium2 Bass kernel for a dense-transformer attention block (v3).

Module: y = o_proj(causal_sdpa(rope(q_proj(x)), rope(k_proj(x)), v_proj(x)))
Shapes: x [2, 2048, 2048], 32 q heads / 8 kv heads, head_dim 64, fp32 I/O.

Sharding (8 NeuronCores): 2-way data parallel over batch x 4-way tensor
parallel over heads. Core c handles batch c//4 and head group c%4
(8 q heads, 2 kv heads). Each core produces a partial [2048, 2048]
output (its heads' slice of o_proj); the host sums the 4 partials per
batch.

v3 changes vs baseline:
- Single software-pipelined phase: attention for query chunk qc starts
  as soon as seq tiles 0..4qc+3 are projected; projection of the next
  window's tiles and o_proj of the previous window fill PE slack while
  the Scalar engine grinds exp.
- Score matmuls are row-tiled: head pair (s, s+4) contracts K=64 on
  partition strips 0:64 / 64:128 concurrently (kv0 lo, kv1 hi), no
  zero-padding, ~2x score throughput.
- q/k heads are pre-paired on the host (column order h, h+4) so one
  [128,128] PE transpose produces both strips of a qT pair slot.
- RoPE runs on bf16 SBUF copies (2x DVE modes) instead of fp32 PSUM.
- Batched evictions / masks / normalize via strided multi-dim APs.
- Output written bf16 (host accumulates in fp32).
"""

import os
import sys
import types

import numpy as np

sys.path.insert(0, "/opt/trn_rl_repo")

import concourse.bacc as bacc  # noqa: E402
import concourse.bass as bass  # noqa: E402
import concourse.tile as tile  # noqa: E402
from concourse import mybir  # noqa: E402
from concourse.bass_utils import run_bass_kernel_spmd  # noqa: E402
from concourse.masks import make_identity  # noqa: E402

try:
    import ml_dtypes
    BF16 = ml_dtypes.bfloat16
except ImportError:  # pragma: no cover
    BF16 = np.dtype("bfloat16")

HIDDEN = 2048
SEQ = 2048
BATCH = 2
N_HEADS = 32
N_KV_HEADS = 8
HEAD_DIM = 64
ROPE_THETA = 10000.0

N_CORES = 8
TP = 4                      # head-parallel ways
QH = N_HEADS // TP          # 8 q heads per core
KVH = N_KV_HEADS // TP      # 2 kv heads per core
KT = HIDDEN // 128          # 16 contraction tiles
TT = SEQ // 128             # 16 seq tiles
F_QKV = QH * HEAD_DIM + 2 * KVH * HEAD_DIM  # 512 + 128 + 128 = 768
F_O = QH * HEAD_DIM         # 512

FP32 = mybir.dt.float32
BF16_DT = mybir.dt.bfloat16


def _build_nc():
    nc = bacc.Bacc("TRN2", target_bir_lowering=False, debug=False)

    xT = nc.dram_tensor("xT", [HIDDEN, SEQ], BF16_DT, kind="ExternalInput")
    wqkv = nc.dram_tensor("wqkv", [HIDDEN, F_QKV], BF16_DT, kind="ExternalInput")
    wo = nc.dram_tensor("wo", [F_O, HIDDEN], BF16_DT, kind="ExternalInput")
    cos = nc.dram_tensor("cos", [SEQ, HEAD_DIM], BF16_DT, kind="ExternalInput")
    sin = nc.dram_tensor("sin", [SEQ, HEAD_DIM], BF16_DT, kind="ExternalInput")
    maskt = nc.dram_tensor("maskt", [128, 128], BF16_DT, kind="ExternalInput")
    out = nc.dram_tensor("out", [SEQ, HIDDEN], BF16_DT, kind="ExternalOutput")

    with tile.TileContext(nc) as tc:
        _emit(nc, tc, xT, wqkv, wo, cos, sin, maskt, out)
    nc.compile()
    return nc


def _bcast(ap, n, axis_pos=1):
    """Insert a step-0 (broadcast) dim of size n into an AP at axis_pos."""
    new = list(ap.ap)
    new.insert(axis_pos, [0, n])
    return bass.AP(tensor=ap.tensor, offset=ap.offset, ap=new)


def _emit(nc, tc, xT, wqkv, wo, cos, sin, maskt, out):
    from contextlib import ExitStack
    ctx = ExitStack()
    Exp = mybir.ActivationFunctionType.Exp
    mult = mybir.AluOpType.mult

    const = ctx.enter_context(tc.tile_pool(name="const", bufs=1))
    persist = ctx.enter_context(tc.tile_pool(name="persist", bufs=1))
    work = ctx.enter_context(tc.tile_pool(name="work", bufs=2))
    att = ctx.enter_context(tc.tile_pool(name="att", bufs=3))
    fwork = ctx.enter_context(tc.tile_pool(name="fwork", bufs=3))
    # PSUM: psS 2x2 banks + psO 2 + psPJ 1 + psAux 1 = 8 banks
    psS = ctx.enter_context(tc.tile_pool(name="psS", bufs=2, space="PSUM"))
    psO = ctx.enter_context(tc.tile_pool(name="psO", bufs=1, space="PSUM"))
    psPJ = ctx.enter_context(tc.tile_pool(name="psPJ", bufs=1, space="PSUM"))
    psAux = ctx.enter_context(tc.tile_pool(name="psAux", bufs=1, space="PSUM"))

    # ---- constants ----
    cos_sb = const.tile([128, TT, HEAD_DIM], BF16_DT)
    sin_sb = const.tile([128, TT, HEAD_DIM], BF16_DT)
    mask_sb = const.tile([128, 128], BF16_DT)
    idn = const.tile([128, 128], BF16_DT)
    make_identity(nc, idn[:])
    wo_sb = const.tile([128, F_O // 128, HIDDEN], BF16_DT)

    # ---- persistent tensors ----
    xT_sb = persist.tile([128, KT, SEQ], BF16_DT)
    w_sb = persist.tile([128, KT, F_QKV], BF16_DT)
    # qT pair slot s: head s on partitions 0:64, head s+4 on 64:128
    qT_sb = persist.tile([128, 4, SEQ], BF16_DT)
    # kT: kv0 on partitions 0:64, kv1 on 64:128
    kT_sb = persist.tile([128, SEQ], BF16_DT)
    # v natural [seq-part, tile, kv, 65] with ones column 64
    v_all = persist.tile([128, TT, KVH, HEAD_DIM + 1], BF16_DT)
    nc.gpsimd.memset(v_all[:, :, :, HEAD_DIM:HEAD_DIM + 1], 1.0)
    # o (normalized attention out, natural layout) [seq-part, tile, feat]
    o_sb = persist.tile([128, TT, F_O], BF16_DT)
    # oT (feature-part) [128, kf, seq]
    oT_sb = persist.tile([128, 4, SEQ], BF16_DT)

    # ---- input DMAs. DMA *issue* is the prologue bottleneck (~0.6us per
    # dma_start, serialized per issuing engine), so split the critical
    # pieces (w + first xT seq-chunk) across the Sync and GpSimd queues
    # and defer everything else to GpSimd. ----
    w_r = wqkv[:].rearrange("(k p) f -> p k f", p=128)
    xT_r = xT[:].rearrange("(k p) t -> p k t", p=128)
    # critical path (prologue psq consumes w[k] + xT[k, 0:512] in k order):
    # batched chunks interleaved across all three queues, ~90GB/s each.
    # q-columns of w in k-quads on sync (matches prologue psq consumption
    # order); xT first chunk split scalar/gpsimd; kv-columns of w and wo
    # deferred (pskv/o_proj consume them later). Keeps the critical path
    # at 4MB instead of 5MB and each piece ~0.5MB (~5.5us/queue).
    for k4 in range(4):
        ks = slice(4 * k4, 4 * k4 + 4)
        nc.sync.dma_start(out=w_sb[:, ks, 0:F_O], in_=w_r[:, ks, 0:F_O])
    nc.scalar.dma_start(out=xT_sb[:, 0:4, 0:512], in_=xT_r[:, 0:4, 0:512])
    nc.gpsimd.dma_start(out=xT_sb[:, 8:16, 0:512], in_=xT_r[:, 8:16, 0:512])
    nc.scalar.dma_start(out=xT_sb[:, 4:8, 0:512], in_=xT_r[:, 4:8, 0:512])
    nc.scalar.dma_start(out=w_sb[:, 0:8, F_O:F_QKV],
                        in_=w_r[:, 0:8, F_O:F_QKV])
    nc.scalar.dma_start(out=w_sb[:, 8:16, F_O:F_QKV],
                        in_=w_r[:, 8:16, F_O:F_QKV])
    nc.sync.dma_start(out=cos_sb[:],
                      in_=cos[:].rearrange("(t p) d -> p t d", p=128))
    nc.sync.dma_start(out=sin_sb[:],
                      in_=sin[:].rearrange("(t p) d -> p t d", p=128))
    nc.sync.dma_start(out=mask_sb[:], in_=maskt[:])
    nc.scalar.dma_start(out=wo_sb[:],
                        in_=wo[:].rearrange("(k p) d -> p k d", p=128))
    engs = {(1, 0): nc.gpsimd, (1, 1): nc.gpsimd,   # chunk 1: needed ~35us
            (2, 0): nc.gpsimd, (2, 1): nc.sync,      # chunk 2: ~80us
            (3, 0): nc.sync, (3, 1): nc.sync}        # chunk 3: ~150us
    for tc4 in range(1, 4):
        csl = bass.ts(tc4, 512)
        for k8 in range(2):
            ks = slice(8 * k8, 8 * k8 + 8)
            engs[(tc4, k8)].dma_start(out=xT_sb[:, ks, csl],
                                      in_=xT_r[:, ks, csl])

    # ---- building blocks ----
    def rope(t, src3, nheads, tag, bufs=None):
        """RoPE on bf16 SBUF tile [128, nheads, 64] -> new tile."""
        dst = work.tile([128, nheads, HEAD_DIM], BF16_DT, tag=tag, name=tag,
                        bufs=bufs)
        cos_t = cos_sb[:, t, :]
        sin_lo = sin_sb[:, t, 0:32]
        sin_hi = sin_sb[:, t, 32:64]
        nc.vector.tensor_tensor(dst[:], src3[:], _bcast(cos_t, nheads), op=mult)
        tmp = work.tile([128, nheads, 32], BF16_DT, tag=tag + "t",
                        name=tag + "t", bufs=bufs)
        nc.vector.tensor_tensor(tmp[:], src3[:, :, 32:64],
                                _bcast(sin_lo, nheads), op=mult)
        nc.vector.tensor_sub(dst[:, :, 0:32], dst[:, :, 0:32], tmp[:])
        nc.vector.tensor_tensor(tmp[:], src3[:, :, 0:32],
                                _bcast(sin_hi, nheads), op=mult)
        nc.vector.tensor_add(dst[:, :, 32:64], dst[:, :, 32:64], tmp[:])
        return dst

    def proj_chunks(t):
        """Emission chunks (closures) projecting q/k/v for seq tile t:
        matmul slices, PSUM evictions, RoPE, then PE transposes into
        qT/kT. Chunks must run in list order."""
        tsl = bass.ts(t, 128)
        box = {}

        def c_psq_alloc():
            box["psq"] = psPJ.tile([128, F_O], FP32, tag="pj", name=f"psq{t}")

        def c_psq(k0):
            def f():
                for k in range(k0, k0 + 4):
                    nc.tensor.matmul(box["psq"][:], xT_sb[:, k, tsl],
                                     w_sb[:, k, 0:F_O],
                                     start=(k == 0), stop=(k == KT - 1))
            return f

        def c_qcopy():
            q_raw = work.tile([128, QH, HEAD_DIM], BF16_DT, tag="qr",
                              name=f"qr{t}")
            nc.vector.tensor_copy(
                q_raw[:], box["psq"][:].rearrange("p (h d) -> p h d", d=HEAD_DIM))
            box["q_raw"] = q_raw
            box["pskv"] = psPJ.tile([128, F_QKV - F_O], FP32, tag="pj",
                                    name=f"pskv{t}")

        def c_pskv(k0):
            def f():
                for k in range(k0, k0 + 4):
                    nc.tensor.matmul(box["pskv"][:], xT_sb[:, k, tsl],
                                     w_sb[:, k, F_O:F_QKV],
                                     start=(k == 0), stop=(k == KT - 1))
            return f

        def c_rope():
            pskv = box["pskv"]
            k_raw = work.tile([128, KVH, HEAD_DIM], BF16_DT, tag="kr",
                              name=f"kr{t}")
            nc.vector.tensor_copy(
                k_raw[:],
                pskv[:, 0:KVH * HEAD_DIM].rearrange("p (h d) -> p h d",
                                                    d=HEAD_DIM))
            nc.vector.tensor_copy(
                v_all[:, t, :, 0:HEAD_DIM],
                pskv[:, KVH * HEAD_DIM:].rearrange("p (h d) -> p h d",
                                                   d=HEAD_DIM))
            box["q_nat"] = rope(t, box["q_raw"], QH, "qn")
            box["k_nat"] = rope(t, k_raw, KVH, "kn")

        def c_tp():
            tp = psAux.tile([128, 4, 128], BF16_DT, tag="po", name=f"tpq{t}")
            for i in range(4):
                # pair (head i, head i+4) packed in q_nat slots 2i, 2i+1
                nc.tensor.transpose(
                    tp[:, i, :],
                    box["q_nat"][:, 2 * i:2 * i + 2, :].rearrange(
                        "p h d -> p (h d)"),
                    idn[:])
            nc.vector.tensor_copy(qT_sb[:, :, tsl], tp[:])
            tpk = psAux.tile([128, 128], BF16_DT, tag="po", name=f"tpk{t}")
            nc.tensor.transpose(
                tpk[:], box["k_nat"][:].rearrange("p h d -> p (h d)"), idn[:])
            nc.vector.tensor_copy(kT_sb[:, tsl], tpk[:])

        ch = [c_psq_alloc, c_psq(0), c_psq(4), c_psq(8), c_psq(12), c_qcopy,
              c_pskv(0), c_pskv(4), c_pskv(8), c_pskv(12), c_rope, c_tp]
        return ch

    def oproj_chunks(t, pool, tag):
        """Emission chunks for o transpose + o_proj + output DMA of tile t."""
        tsl = bass.ts(t, 128)
        box = {}

        def c_tp2():
            tp2 = psAux.tile([128, 4, 128], BF16_DT, tag="po", name=f"tpo{t}")
            for kf in range(4):
                nc.tensor.transpose(tp2[:, kf, :],
                                    o_sb[:, t, bass.ts(kf, 128)], idn[:])
            nc.vector.tensor_copy(oT_sb[:, :, tsl], tp2[:])
            box["ost"] = fwork.tile([128, 4, 512], BF16_DT, tag="ost",
                                    name=f"ost{t}", bufs=4)

        def c_po(nch):
            def f():
                po = pool.tile([128, 512], FP32, tag=tag, name=f"po{t}_{nch}")
                for kf in range(4):
                    nc.tensor.matmul(po[:], oT_sb[:, kf, tsl],
                                     wo_sb[:, kf, bass.ts(nch, 512)],
                                     start=(kf == 0), stop=(kf == 3))
                nc.vector.tensor_copy(box["ost"][:, nch, :], po[:])
                eng = (nc.sync, nc.scalar, nc.gpsimd)[(t + nch) % 3]
                eng.dma_start(out=out[tsl, bass.ts(nch, 512)],
                              in_=box["ost"][:, nch, :])
            return f

        return [c_tp2, c_po(0), c_po(1), c_po(2), c_po(3)]

    def emit_scores(qc, s, ik):
        """Score pair (row-tiled lo/hi strips) + exp + mask; returns
        (p_sb, j0) for the lagged O matmuls."""
        j0 = max(0, ik - 4 * qc)
        c0 = j0 * 128
        qbase = qc * 512
        ksl = bass.ts(ik, 128)
        stp = psS.tile([128, 2, 512], FP32, tag="st", name=f"st{qc}_{s}_{ik}")
        nc.tensor.matmul(stp[:, 0, c0:512], kT_sb[0:64, ksl],
                         qT_sb[0:64, s, qbase + c0:qbase + 512],
                         start=True, stop=True)
        nc.tensor.matmul(stp[:, 1, c0:512], kT_sb[64:128, ksl],
                         qT_sb[64:128, s, qbase + c0:qbase + 512],
                         start=True, stop=True)
        p_sb = att.tile([128, 2, 512], BF16_DT, tag="p", name=f"p{qc}_{s}_{ik}")
        nc.scalar.activation(p_sb[:, :, c0:512], stp[:, :, c0:512],
                             Exp, scale=0.125)
        return p_sb, j0

    def emit_o_mms(qc, psOt, ik, p_sb, j0):
        if ik >= 4 * qc:  # diagonal tile: mask both strips in one op
            nc.vector.tensor_tensor(
                p_sb[:, :, bass.ts(j0, 128)], p_sb[:, :, bass.ts(j0, 128)],
                _bcast(mask_sb[:], 2), op=mult)
        for m in range(2):
            for j in range(j0, 4):
                nc.tensor.matmul(
                    psOt[:, m, bass.ds(j * 68, HEAD_DIM + 1)],
                    p_sb[:, m, bass.ts(j, 128)],
                    v_all[:, ik, m, :],
                    start=(ik == 0 and j == 0),
                    stop=(ik == 4 * qc + j),
                    skip_group_check=(j > 0))

    def emit_normalize(qc, s, psOt):
        """One reciprocal + one TT covering both heads x 4 query tiles."""
        g = psOt[:, :, 0:272].rearrange("p m (j x) -> p m j x", x=68)
        rc = fwork.tile([128, 2, 4], FP32, tag="rc", name=f"rc{qc}_{s}")
        nc.vector.reciprocal(
            rc[:], g[:, :, :, 64:65].rearrange("p m j x -> p m (j x)"))
        o_ap = bass.AP(
            tensor=o_sb[:].tensor,
            offset=o_sb[:].offset + (4 * qc) * F_O + s * HEAD_DIM,
            ap=[list(o_sb[:].ap[0]), [4 * HEAD_DIM, 2], [F_O, 4],
                [1, HEAD_DIM]])
        nc.vector.tensor_tensor(o_ap, g[:, :, :, 0:HEAD_DIM],
                                _bcast(rc[:], HEAD_DIM, axis_pos=3), op=mult)

    def window(qc, fillers):
        """One query-chunk window: the (s, ik) score/exp spine with O
        matmuls lagging one slot and filler chunks spread across slots."""
        n_ik = 4 * qc + 4
        spine = [(s, ik) for s in range(4) for ik in range(n_ik)]
        nslots = len(spine)
        sched = [[] for _ in range(nslots)]
        for i, f in enumerate(fillers):
            sched[i * nslots // len(fillers)].append(f)
        pend = None        # (s, ik, p_sb, j0, psOt)
        psOt = None
        for idx, (s, ik) in enumerate(spine):
            p_sb, j0 = emit_scores(qc, s, ik)
            if pend is not None:
                ps_, pik, pp, pj0, pO = pend
                emit_o_mms(qc, pO, pik, pp, pj0)
                if pik == n_ik - 1:  # pair ps_ finished
                    emit_normalize(qc, ps_, pO)
            if ik == 0:  # new accumulator after the old pair's flush
                psOt = psO.tile([128, 2, 512], FP32, tag="O",
                                name=f"O{qc}_{s}")
            pend = (s, ik, p_sb, j0, psOt)
            for f in sched[idx]:
                f()
        ps_, pik, pp, pj0, pO = pend
        emit_o_mms(qc, pO, pik, pp, pj0)
        emit_normalize(qc, ps_, pO)

    # ---- pipelined schedule ----
    # prologue: tiles 0..3 projected with 4 concurrent PSUM accumulators
    # (psPJ + psAux + both psS slots, all idle before window 0), k-major so
    # each arriving xT/w DMA piece immediately feeds 4 tiles of matmuls.
    warm = fwork.tile([128, 8], FP32, tag="rc", name="warm")
    nc.scalar.activation(warm[:], warm[:], Exp, scale=1.0)  # ACT table load
    slot = [(psPJ, "pj"), (psAux, "po"), (psS, "st"), (psS, "st")]
    pro_q = [slot[t][0].tile([128, F_O], FP32, tag=slot[t][1], name=f"pq{t}")
             for t in range(4)]
    for k4 in range(4):
        for t in range(4):
            for k in range(4 * k4, 4 * k4 + 4):
                nc.tensor.matmul(pro_q[t][:], xT_sb[:, k, bass.ts(t, 128)],
                                 w_sb[:, k, 0:F_O],
                                 start=(k == 0), stop=(k == KT - 1))
    q_raws = []
    for t in range(4):
        q_raw = work.tile([128, QH, HEAD_DIM], BF16_DT, tag="qrp",
                          name=f"qrp{t}", bufs=4)
        nc.vector.tensor_copy(
            q_raw[:], pro_q[t][:].rearrange("p (h d) -> p h d", d=HEAD_DIM))
        q_raws.append(q_raw)
    pro_kv = [slot[t][0].tile([128, F_QKV - F_O], FP32, tag=slot[t][1],
                              name=f"pkv{t}") for t in range(4)]
    for k4 in range(4):
        for t in range(4):
            for k in range(4 * k4, 4 * k4 + 4):
                nc.tensor.matmul(pro_kv[t][:], xT_sb[:, k, bass.ts(t, 128)],
                                 w_sb[:, k, F_O:F_QKV],
                                 start=(k == 0), stop=(k == KT - 1))
    nats = []
    for t in range(4):
        k_raw = work.tile([128, KVH, HEAD_DIM], BF16_DT, tag="krp",
                          name=f"krp{t}", bufs=4)
        nc.vector.tensor_copy(
            k_raw[:],
            pro_kv[t][:, 0:KVH * HEAD_DIM].rearrange("p (h d) -> p h d",
                                                     d=HEAD_DIM))
        nc.vector.tensor_copy(
            v_all[:, t, :, 0:HEAD_DIM],
            pro_kv[t][:, KVH * HEAD_DIM:].rearrange("p (h d) -> p h d",
                                                    d=HEAD_DIM))
        nats.append((rope(t, q_raws[t], QH, f"qp{t}", bufs=1),
                     rope(t, k_raw, KVH, f"kp{t}", bufs=1)))
    for t in range(4):
        q_nat, k_nat = nats[t]
        tsl = bass.ts(t, 128)
        tp = psAux.tile([128, 4, 128], BF16_DT, tag="po", name=f"ptpq{t}")
        for i in range(4):
            nc.tensor.transpose(
                tp[:, i, :],
                q_nat[:, 2 * i:2 * i + 2, :].rearrange("p h d -> p (h d)"),
                idn[:])
        nc.vector.tensor_copy(qT_sb[:, :, tsl], tp[:])
        tpk = psAux.tile([128, 128], BF16_DT, tag="po", name=f"ptpk{t}")
        nc.tensor.transpose(
            tpk[:], k_nat[:].rearrange("p h d -> p (h d)"), idn[:])
        nc.vector.tensor_copy(kT_sb[:, tsl], tpk[:])

    # window qc must fully project tiles 4qc+4..4qc+7 (read by window qc+1)
    w_fill = {
        0: [c for t in (4, 5, 6, 7) for c in proj_chunks(t)],
        1: ([c for t in (8, 9, 10, 11) for c in proj_chunks(t)]
            + [c for t in (0, 1) for c in oproj_chunks(t, psAux, "po")]),
        2: ([c for t in (12, 13, 14, 15) for c in proj_chunks(t)]
            + [c for t in (2, 3, 4, 5) for c in oproj_chunks(t, psAux, "po")]),
        3: [c for t in (6, 7, 8, 9, 10, 11)
            for c in oproj_chunks(t, psAux, "po")],
    }
    for qc in range(4):
        window(qc, w_fill[qc])
    # tail: last window's o_proj, pipelined over psAux + idle psS slots
    tail = []
    for i, t in enumerate(range(12, 16)):
        tail.append(oproj_chunks(t, psAux if i % 3 == 0 else psS,
                                 "po" if i % 3 == 0 else "st"))
    for group in zip(*tail):  # interleave the 4 tiles' chunks
        for c in group:
            c()
    ctx.close()


_NC_CACHE = None


def _get_nc():
    global _NC_CACHE
    if _NC_CACHE is None:
        _NC_CACHE = _build_nc()
    return _NC_CACHE


def _rope_tables(pos):
    pos = np.asarray(pos, dtype=np.float32)  # [SEQ]
    inv = (1.0 / (np.float32(ROPE_THETA)
                  ** (np.arange(0, HEAD_DIM, 2, dtype=np.float32)
                      / np.float32(HEAD_DIM)))).astype(np.float32)
    fr = pos[:, None] * inv[None, :]                       # [SEQ, 32]
    emb = np.concatenate([fr, fr], axis=-1).astype(np.float32)
    return np.cos(emb).astype(BF16), np.sin(emb).astype(BF16)


def _make_in_maps(input_ids, Wq, Wk, Wv, Wo, position_ids):
    x = np.asarray(input_ids, dtype=np.float32)
    Wq = np.asarray(Wq, dtype=np.float32)
    Wk = np.asarray(Wk, dtype=np.float32)
    Wv = np.asarray(Wv, dtype=np.float32)
    Wo = np.asarray(Wo, dtype=np.float32)
    pos = np.asarray(position_ids)

    maskt = np.triu(np.ones((128, 128), dtype=np.float32)).astype(BF16)

    in_maps = []
    for c in range(N_CORES):
        b, g = c // TP, c % TP
        xTc = np.ascontiguousarray(x[b].T).astype(BF16)
        wq = Wq[:, g * QH * HEAD_DIM:(g + 1) * QH * HEAD_DIM]
        # pair-interleave q head columns: (0,4),(1,5),(2,6),(3,7)
        wq4 = wq.reshape(HIDDEN, QH, HEAD_DIM)
        order = [0, 4, 1, 5, 2, 6, 3, 7]
        wq = wq4[:, order, :].reshape(HIDDEN, QH * HEAD_DIM)
        wk = Wk[:, g * KVH * HEAD_DIM:(g + 1) * KVH * HEAD_DIM]
        wv = Wv[:, g * KVH * HEAD_DIM:(g + 1) * KVH * HEAD_DIM]
        wqkv = np.concatenate([wq, wk, wv], axis=1).astype(BF16)
        wo_s = np.ascontiguousarray(
            Wo[g * F_O:(g + 1) * F_O, :]).astype(BF16)
        cos_t, sin_t = _rope_tables(pos[b])
        in_maps.append({
            "xT": xTc,
            "wqkv": np.ascontiguousarray(wqkv),
            "wo": wo_s,
            "cos": cos_t,
            "sin": sin_t,
            "maskt": maskt,
        })
    return in_maps


def _run(in_maps, trace=False):
    nc = _get_nc()
    kwargs = {}
    if trace:
        _install_profile_hook()
        kwargs["trace"] = True
    return run_bass_kernel_spmd(nc, in_maps, core_ids=list(range(N_CORES)),
                                **kwargs)


def _install_profile_hook():
    """This image's antenv lacks axon_hooks; register the NTFF profile hook
    manually so trace=True yields hardware exec times."""
    if "antenv.axon_hooks" in sys.modules:
        return
    import antenv
    mod = types.ModuleType("antenv.axon_hooks")
    state = {"hook": None}
    mod.set_axon_ntff_profile_hook = lambda h: state.__setitem__("hook", h)
    mod.get_axon_ntff_profile_hook = lambda: state["hook"]
    sys.modules["antenv.axon_hooks"] = mod
    antenv.axon_hooks = mod
    try:
        from trn_agent_boot.trn_boot import _ntff_profile_via_ctypes
        mod.set_axon_ntff_profile_hook(
            _ntff_profile_via_ctypes("/opt/axon/libaxon_pjrt.so"))
    except Exception:
        pass


def kernel(input_ids, Wq, Wk, Wv, Wo, position_ids):
    in_maps = _make_in_maps(input_ids, Wq, Wk, Wv, Wo, position_ids)
    res = _run(in_maps, trace=bool(os.environ.get("KERNEL_TRACE")))
    if os.environ.get("KERNEL_TRACE"):
        print(f"HW exec time: {res.exec_time_ns} ns "
              f"(mean {res.mean_exec_time_ns})")
    # undo the host-side q-head pairing: o_sb feature order is natural
    # h*64+d with h = local head index, same as Wo rows -> nothing to undo.
    out = np.zeros((BATCH, SEQ, HIDDEN), dtype=np.float32)
    for c in range(N_CORES):
        out[c // TP] += res.results[c]["out"].astype(np.float32)
    return out



# revision 42
# speedup vs baseline: 1.0098x; 1.0098x over previous
"""Trainium2 Bass kernel for a dense-transformer attention block (v3).

Module: y = o_proj(causal_sdpa(rope(q_proj(x)), rope(k_proj(x)), v_proj(x)))
Shapes: x [2, 2048, 2048], 32 q heads / 8 kv heads, head_dim 64, fp32 I/O.

Sharding (8 NeuronCores): 2-way data parallel over batch x 4-way tensor
parallel over heads. Core c handles batch c//4 and head group c%4
(8 q heads, 2 kv heads). Each core produces a partial [2048, 2048]
output (its heads' slice of o_proj); the host sums the 4 partials per
batch.

v3 changes vs baseline:
- Single software-pipelined phase: attention for query chunk qc starts
  as soon as seq tiles 0..4qc+3 are projected; projection of the next
  window's tiles and o_proj of the previous window fill PE slack while
  the Scalar engine grinds exp.
- Score matmuls are row-tiled: head pair (s, s+4) contracts K=64 on
  partition strips 0:64 / 64:128 concurrently (kv0 lo, kv1 hi), no
  zero-padding, ~2x score throughput.
- q/k heads are pre-paired on the host (column order h, h+4) so one
  [128,128] PE transpose produces both strips of a qT pair slot.
- RoPE runs on bf16 SBUF copies (2x DVE modes) instead of fp32 PSUM.
- Batched evictions / masks / normalize via strided multi-dim APs.
- Output written bf16 (host accumulates in fp32).
"""

import os
import sys
import types

import numpy as np

sys.path.insert(0, "/opt/trn_rl_repo")

import concourse.bacc as bacc  # noqa: E402
import concourse.bass as bass  # noqa: E402
import concourse.tile as tile  # noqa: E402
from concourse import mybir  # noqa: E402
from concourse.bass_utils import run_bass_kernel_spmd  # noqa: E402
from concourse.masks import make_identity  # noqa: E402

try:
    import ml_dtypes
    BF16 = ml_dtypes.bfloat16
except ImportError:  # pragma: no cover
    BF16 = np.dtype("bfloat16")

HIDDEN = 2048
SEQ = 2048
BATCH = 2
N_HEADS = 32
N_KV_HEADS = 8
HEAD_DIM = 64
ROPE_THETA = 10000.0

N_CORES = 8
TP = 4                      # head-parallel ways
QH = N_HEADS // TP          # 8 q heads per core
KVH = N_KV_HEADS // TP      # 2 kv heads per core
KT = HIDDEN // 128          # 16 contraction tiles
TT = SEQ // 128             # 16 seq tiles
F_QKV = QH * HEAD_DIM + 2 * KVH * HEAD_DIM  # 512 + 128 + 128 = 768
F_O = QH * HEAD_DIM         # 512

FP32 = mybir.dt.float32
BF16_DT = mybir.dt.bfloat16


def _build_nc():
    nc = bacc.Bacc("TRN2", target_bir_lowering=False, debug=False)

    xT = nc.dram_tensor("xT", [HIDDEN, SEQ], BF16_DT, kind="ExternalInput")
    wqkv = nc.dram_tensor("wqkv", [HIDDEN, F_QKV], BF16_DT, kind="ExternalInput")
    wo = nc.dram_tensor("wo", [F_O, HIDDEN], BF16_DT, kind="ExternalInput")
    cos = nc.dram_tensor("cos", [SEQ, HEAD_DIM], BF16_DT, kind="ExternalInput")
    sin = nc.dram_tensor("sin", [SEQ, HEAD_DIM], BF16_DT, kind="ExternalInput")
    maskt = nc.dram_tensor("maskt", [128, 128], BF16_DT, kind="ExternalInput")
    out = nc.dram_tensor("out", [SEQ, HIDDEN], BF16_DT, kind="ExternalOutput")

    with tile.TileContext(nc) as tc:
        _emit(nc, tc, xT, wqkv, wo, cos, sin, maskt, out)
    nc.compile()
    return nc


def _bcast(ap, n, axis_pos=1):
    """Insert a step-0 (broadcast) dim of size n into an AP at axis_pos."""
    new = list(ap.ap)
    new.insert(axis_pos, [0, n])
    return bass.AP(tensor=ap.tensor, offset=ap.offset, ap=new)


def _emit(nc, tc, xT, wqkv, wo, cos, sin, maskt, out):
    from contextlib import ExitStack
    ctx = ExitStack()
    Exp = mybir.ActivationFunctionType.Exp
    mult = mybir.AluOpType.mult

    const = ctx.enter_context(tc.tile_pool(name="const", bufs=1))
    persist = ctx.enter_context(tc.tile_pool(name="persist", bufs=1))
    work = ctx.enter_context(tc.tile_pool(name="work", bufs=2))
    att = ctx.enter_context(tc.tile_pool(name="att", bufs=3))
    fwork = ctx.enter_context(tc.tile_pool(name="fwork", bufs=3))
    # PSUM: psS 2x2 banks + psO 2 + psPJ 1 + psAux 1 = 8 banks
    psS = ctx.enter_context(tc.tile_pool(name="psS", bufs=2, space="PSUM"))
    psO = ctx.enter_context(tc.tile_pool(name="psO", bufs=1, space="PSUM"))
    psPJ = ctx.enter_context(tc.tile_pool(name="psPJ", bufs=1, space="PSUM"))
    psAux = ctx.enter_context(tc.tile_pool(name="psAux", bufs=1, space="PSUM"))

    # ---- constants ----
    cos_sb = const.tile([128, TT, HEAD_DIM], BF16_DT)
    sin_sb = const.tile([128, TT, HEAD_DIM], BF16_DT)
    mask_sb = const.tile([128, 128], BF16_DT)
    idn = const.tile([128, 128], BF16_DT)
    make_identity(nc, idn[:])
    wo_sb = const.tile([128, F_O // 128, HIDDEN], BF16_DT)

    # ---- persistent tensors ----
    xT_sb = persist.tile([128, KT, SEQ], BF16_DT)
    w_sb = persist.tile([128, KT, F_QKV], BF16_DT)
    # qT pair slot s: head s on partitions 0:64, head s+4 on 64:128
    qT_sb = persist.tile([128, 4, SEQ], BF16_DT)
    # kT: kv0 on partitions 0:64, kv1 on 64:128
    kT_sb = persist.tile([128, SEQ], BF16_DT)
    # v natural [seq-part, tile, kv, 65] with ones column 64
    v_all = persist.tile([128, TT, KVH, HEAD_DIM + 1], BF16_DT)
    nc.gpsimd.memset(v_all[:, :, :, HEAD_DIM:HEAD_DIM + 1], 1.0)
    # o (normalized attention out, natural layout) [seq-part, tile, feat]
    o_sb = persist.tile([128, TT, F_O], BF16_DT)
    # oT (feature-part) [128, kf, seq]
    oT_sb = persist.tile([128, 4, SEQ], BF16_DT)

    # ---- input DMAs. DMA *issue* is the prologue bottleneck (~0.6us per
    # dma_start, serialized per issuing engine), so split the critical
    # pieces (w + first xT seq-chunk) across the Sync and GpSimd queues
    # and defer everything else to GpSimd. ----
    w_r = wqkv[:].rearrange("(k p) f -> p k f", p=128)
    xT_r = xT[:].rearrange("(k p) t -> p k t", p=128)
    # critical path (prologue psq consumes w[k] + xT[k, 0:512] in k order):
    # batched chunks interleaved across all three queues, ~90GB/s each.
    nc.sync.dma_start(out=w_sb[:, 0:4, :], in_=w_r[:, 0:4, :])
    nc.scalar.dma_start(out=xT_sb[:, 0:4, 0:512], in_=xT_r[:, 0:4, 0:512])
    nc.gpsimd.dma_start(out=xT_sb[:, 8:16, 0:512], in_=xT_r[:, 8:16, 0:512])
    nc.sync.dma_start(out=w_sb[:, 4:8, :], in_=w_r[:, 4:8, :])
    nc.scalar.dma_start(out=xT_sb[:, 4:8, 0:512], in_=xT_r[:, 4:8, 0:512])
    nc.scalar.dma_start(out=w_sb[:, 8:12, :], in_=w_r[:, 8:12, :])
    nc.scalar.dma_start(out=w_sb[:, 12:16, :], in_=w_r[:, 12:16, :])
    nc.sync.dma_start(out=cos_sb[:],
                      in_=cos[:].rearrange("(t p) d -> p t d", p=128))
    nc.sync.dma_start(out=sin_sb[:],
                      in_=sin[:].rearrange("(t p) d -> p t d", p=128))
    nc.sync.dma_start(out=mask_sb[:], in_=maskt[:])
    nc.scalar.dma_start(out=wo_sb[:],
                        in_=wo[:].rearrange("(k p) d -> p k d", p=128))
    engs = {(1, 0): nc.gpsimd, (1, 1): nc.gpsimd,   # chunk 1: needed ~20us
            (2, 0): nc.gpsimd, (2, 1): nc.sync,      # chunk 2: ~80us
            (3, 0): nc.sync, (3, 1): nc.sync}        # chunk 3: ~150us
    for tc4 in range(1, 4):
        csl = bass.ts(tc4, 512)
        for k8 in range(2):
            ks = slice(8 * k8, 8 * k8 + 8)
            engs[(tc4, k8)].dma_start(out=xT_sb[:, ks, csl],
                                      in_=xT_r[:, ks, csl])

    # ---- building blocks ----
    def rope(t, src3, nheads, tag, bufs=None):
        """RoPE on bf16 SBUF tile [128, nheads, 64] -> new tile."""
        dst = work.tile([128, nheads, HEAD_DIM], BF16_DT, tag=tag, name=tag,
                        bufs=bufs)
        cos_t = cos_sb[:, t, :]
        sin_lo = sin_sb[:, t, 0:32]
        sin_hi = sin_sb[:, t, 32:64]
        nc.vector.tensor_tensor(dst[:], src3[:], _bcast(cos_t, nheads), op=mult)
        tmp = work.tile([128, nheads, 32], BF16_DT, tag=tag + "t",
                        name=tag + "t", bufs=bufs)
        nc.vector.tensor_tensor(tmp[:], src3[:, :, 32:64],
                                _bcast(sin_lo, nheads), op=mult)
        nc.vector.tensor_sub(dst[:, :, 0:32], dst[:, :, 0:32], tmp[:])
        nc.vector.tensor_tensor(tmp[:], src3[:, :, 0:32],
                                _bcast(sin_hi, nheads), op=mult)
        nc.vector.tensor_add(dst[:, :, 32:64], dst[:, :, 32:64], tmp[:])
        return dst

    def proj_chunks(t):
        """Emission chunks (closures) projecting q/k/v for seq tile t:
        matmul slices, PSUM evictions, RoPE, then PE transposes into
        qT/kT. Chunks must run in list order."""
        tsl = bass.ts(t, 128)
        box = {}

        def c_psq_alloc():
            box["psq"] = psPJ.tile([128, F_O], FP32, tag="pj", name=f"psq{t}")

        def c_psq(k0):
            def f():
                for k in range(k0, k0 + 4):
                    nc.tensor.matmul(box["psq"][:], xT_sb[:, k, tsl],
                                     w_sb[:, k, 0:F_O],
                                     start=(k == 0), stop=(k == KT - 1))
            return f

        def c_qcopy():
            q_raw = work.tile([128, QH, HEAD_DIM], BF16_DT, tag="qr",
                              name=f"qr{t}")
            nc.vector.tensor_copy(
                q_raw[:], box["psq"][:].rearrange("p (h d) -> p h d", d=HEAD_DIM))
            box["q_raw"] = q_raw
            box["pskv"] = psPJ.tile([128, F_QKV - F_O], FP32, tag="pj",
                                    name=f"pskv{t}")

        def c_pskv(k0):
            def f():
                for k in range(k0, k0 + 4):
                    nc.tensor.matmul(box["pskv"][:], xT_sb[:, k, tsl],
                                     w_sb[:, k, F_O:F_QKV],
                                     start=(k == 0), stop=(k == KT - 1))
            return f

        def c_rope():
            pskv = box["pskv"]
            k_raw = work.tile([128, KVH, HEAD_DIM], BF16_DT, tag="kr",
                              name=f"kr{t}")
            nc.vector.tensor_copy(
                k_raw[:],
                pskv[:, 0:KVH * HEAD_DIM].rearrange("p (h d) -> p h d",
                                                    d=HEAD_DIM))
            nc.vector.tensor_copy(
                v_all[:, t, :, 0:HEAD_DIM],
                pskv[:, KVH * HEAD_DIM:].rearrange("p (h d) -> p h d",
                                                   d=HEAD_DIM))
            box["q_nat"] = rope(t, box["q_raw"], QH, "qn")
            box["k_nat"] = rope(t, k_raw, KVH, "kn")

        def c_tp():
            tp = psAux.tile([128, 4, 128], BF16_DT, tag="po", name=f"tpq{t}")
            for i in range(4):
                # pair (head i, head i+4) packed in q_nat slots 2i, 2i+1
                nc.tensor.transpose(
                    tp[:, i, :],
                    box["q_nat"][:, 2 * i:2 * i + 2, :].rearrange(
                        "p h d -> p (h d)"),
                    idn[:])
            nc.vector.tensor_copy(qT_sb[:, :, tsl], tp[:])
            tpk = psAux.tile([128, 128], BF16_DT, tag="po", name=f"tpk{t}")
            nc.tensor.transpose(
                tpk[:], box["k_nat"][:].rearrange("p h d -> p (h d)"), idn[:])
            nc.vector.tensor_copy(kT_sb[:, tsl], tpk[:])

        ch = [c_psq_alloc, c_psq(0), c_psq(4), c_psq(8), c_psq(12), c_qcopy,
              c_pskv(0), c_pskv(4), c_pskv(8), c_pskv(12), c_rope, c_tp]
        return ch

    def oproj_chunks(t, pool, tag):
        """Emission chunks for o transpose + o_proj + output DMA of tile t."""
        tsl = bass.ts(t, 128)
        box = {}

        def c_tp2():
            tp2 = psAux.tile([128, 4, 128], BF16_DT, tag="po", name=f"tpo{t}")
            for kf in range(4):
                nc.tensor.transpose(tp2[:, kf, :],
                                    o_sb[:, t, bass.ts(kf, 128)], idn[:])
            nc.vector.tensor_copy(oT_sb[:, :, tsl], tp2[:])
            box["ost"] = fwork.tile([128, 4, 512], BF16_DT, tag="ost",
                                    name=f"ost{t}", bufs=4)

        def c_po(nch):
            def f():
                po = pool.tile([128, 512], FP32, tag=tag, name=f"po{t}_{nch}")
                for kf in range(4):
                    nc.tensor.matmul(po[:], oT_sb[:, kf, tsl],
                                     wo_sb[:, kf, bass.ts(nch, 512)],
                                     start=(kf == 0), stop=(kf == 3))
                nc.vector.tensor_copy(box["ost"][:, nch, :], po[:])
                eng = (nc.sync, nc.scalar, nc.gpsimd)[(t + nch) % 3]
                eng.dma_start(out=out[tsl, bass.ts(nch, 512)],
                              in_=box["ost"][:, nch, :])
            return f

        return [c_tp2, c_po(0), c_po(1), c_po(2), c_po(3)]

    def emit_scores(qc, s, ik):
        """Score pair (row-tiled lo/hi strips) + exp + mask; returns
        (p_sb, j0) for the lagged O matmuls."""
        j0 = max(0, ik - 4 * qc)
        c0 = j0 * 128
        qbase = qc * 512
        ksl = bass.ts(ik, 128)
        stp = psS.tile([128, 2, 512], FP32, tag="st", name=f"st{qc}_{s}_{ik}")
        nc.tensor.matmul(stp[:, 0, c0:512], kT_sb[0:64, ksl],
                         qT_sb[0:64, s, qbase + c0:qbase + 512],
                         start=True, stop=True)
        nc.tensor.matmul(stp[:, 1, c0:512], kT_sb[64:128, ksl],
                         qT_sb[64:128, s, qbase + c0:qbase + 512],
                         start=True, stop=True)
        p_sb = att.tile([128, 2, 512], BF16_DT, tag="p", name=f"p{qc}_{s}_{ik}")
        nc.scalar.activation(p_sb[:, :, c0:512], stp[:, :, c0:512],
                             Exp, scale=0.125)
        return p_sb, j0

    def emit_o_mms(qc, psOt, ik, p_sb, j0):
        if ik >= 4 * qc:  # diagonal tile: mask both strips in one op
            nc.vector.tensor_tensor(
                p_sb[:, :, bass.ts(j0, 128)], p_sb[:, :, bass.ts(j0, 128)],
                _bcast(mask_sb[:], 2), op=mult)
        for m in range(2):
            for j in range(j0, 4):
                nc.tensor.matmul(
                    psOt[:, m, bass.ds(j * 68, HEAD_DIM + 1)],
                    p_sb[:, m, bass.ts(j, 128)],
                    v_all[:, ik, m, :],
                    start=(ik == 0 and j == 0),
                    stop=(ik == 4 * qc + j),
                    skip_group_check=(j > 0))

    def emit_normalize(qc, s, psOt):
        """One reciprocal + one TT covering both heads x 4 query tiles."""
        g = psOt[:, :, 0:272].rearrange("p m (j x) -> p m j x", x=68)
        rc = fwork.tile([128, 2, 4], FP32, tag="rc", name=f"rc{qc}_{s}")
        nc.vector.reciprocal(
            rc[:], g[:, :, :, 64:65].rearrange("p m j x -> p m (j x)"))
        o_ap = bass.AP(
            tensor=o_sb[:].tensor,
            offset=o_sb[:].offset + (4 * qc) * F_O + s * HEAD_DIM,
            ap=[list(o_sb[:].ap[0]), [4 * HEAD_DIM, 2], [F_O, 4],
                [1, HEAD_DIM]])
        nc.vector.tensor_tensor(o_ap, g[:, :, :, 0:HEAD_DIM],
                                _bcast(rc[:], HEAD_DIM, axis_pos=3), op=mult)

    def window(qc, fillers):
        """One query-chunk window: the (s, ik) score/exp spine with O
        matmuls lagging one slot and filler chunks spread across slots."""
        n_ik = 4 * qc + 4
        spine = [(s, ik) for s in range(4) for ik in range(n_ik)]
        nslots = len(spine)
        sched = [[] for _ in range(nslots)]
        for i, f in enumerate(fillers):
            sched[i * nslots // len(fillers)].append(f)
        pend = None        # (s, ik, p_sb, j0, psOt)
        psOt = None
        for idx, (s, ik) in enumerate(spine):
            p_sb, j0 = emit_scores(qc, s, ik)
            if pend is not None:
                ps_, pik, pp, pj0, pO = pend
                emit_o_mms(qc, pO, pik, pp, pj0)
                if pik == n_ik - 1:  # pair ps_ finished
                    emit_normalize(qc, ps_, pO)
            if ik == 0:  # new accumulator after the old pair's flush
                psOt = psO.tile([128, 2, 512], FP32, tag="O",
                                name=f"O{qc}_{s}")
            pend = (s, ik, p_sb, j0, psOt)
            for f in sched[idx]:
                f()
        ps_, pik, pp, pj0, pO = pend
        emit_o_mms(qc, pO, pik, pp, pj0)
        emit_normalize(qc, ps_, pO)

    # ---- pipelined schedule ----
    # prologue: tiles 0..3 projected with 4 concurrent PSUM accumulators
    # (psPJ + psAux + both psS slots, all idle before window 0), k-major so
    # each arriving xT/w DMA piece immediately feeds 4 tiles of matmuls.
    warm = fwork.tile([128, 8], FP32, tag="rc", name="warm")
    nc.scalar.activation(warm[:], warm[:], Exp, scale=1.0)  # ACT table load
    slot = [(psPJ, "pj"), (psAux, "po"), (psS, "st"), (psS, "st")]
    pro_q = [slot[t][0].tile([128, F_O], FP32, tag=slot[t][1], name=f"pq{t}")
             for t in range(4)]
    for k4 in range(4):
        for t in range(4):
            for k in range(4 * k4, 4 * k4 + 4):
                nc.tensor.matmul(pro_q[t][:], xT_sb[:, k, bass.ts(t, 128)],
                                 w_sb[:, k, 0:F_O],
                                 start=(k == 0), stop=(k == KT - 1))
    q_raws = []
    for t in range(4):
        q_raw = work.tile([128, QH, HEAD_DIM], BF16_DT, tag="qrp",
                          name=f"qrp{t}", bufs=4)
        nc.vector.tensor_copy(
            q_raw[:], pro_q[t][:].rearrange("p (h d) -> p h d", d=HEAD_DIM))
        q_raws.append(q_raw)
    pro_kv = [slot[t][0].tile([128, F_QKV - F_O], FP32, tag=slot[t][1],
                              name=f"pkv{t}") for t in range(4)]
    for k4 in range(4):
        for t in range(4):
            for k in range(4 * k4, 4 * k4 + 4):
                nc.tensor.matmul(pro_kv[t][:], xT_sb[:, k, bass.ts(t, 128)],
                                 w_sb[:, k, F_O:F_QKV],
                                 start=(k == 0), stop=(k == KT - 1))
    nats = []
    for t in range(4):
        k_raw = work.tile([128, KVH, HEAD_DIM], BF16_DT, tag="krp",
                          name=f"krp{t}", bufs=4)
        nc.vector.tensor_copy(
            k_raw[:],
            pro_kv[t][:, 0:KVH * HEAD_DIM].rearrange("p (h d) -> p h d",
                                                     d=HEAD_DIM))
        nc.vector.tensor_copy(
            v_all[:, t, :, 0:HEAD_DIM],
            pro_kv[t][:, KVH * HEAD_DIM:].rearrange("p (h d) -> p h d",
                                                    d=HEAD_DIM))
        nats.append((rope(t, q_raws[t], QH, f"qp{t}", bufs=1),
                     rope(t, k_raw, KVH, f"kp{t}", bufs=1)))
    for t in range(4):
        q_nat, k_nat = nats[t]
        tsl = bass.ts(t, 128)
        tp = psAux.tile([128, 4, 128], BF16_DT, tag="po", name=f"ptpq{t}")
        for i in range(4):
            nc.tensor.transpose(
                tp[:, i, :],
                q_nat[:, 2 * i:2 * i + 2, :].rearrange("p h d -> p (h d)"),
                idn[:])
        nc.vector.tensor_copy(qT_sb[:, :, tsl], tp[:])
        tpk = psAux.tile([128, 128], BF16_DT, tag="po", name=f"ptpk{t}")
        nc.tensor.transpose(
            tpk[:], k_nat[:].rearrange("p h d -> p (h d)"), idn[:])
        nc.vector.tensor_copy(kT_sb[:, tsl], tpk[:])

    # window qc must fully project tiles 4qc+4..4qc+7 (read by window qc+1)
    w_fill = {
        0: [c for t in (4, 5, 6, 7) for c in proj_chunks(t)],
        1: ([c for t in (8, 9, 10, 11) for c in proj_chunks(t)]
            + [c for t in (0, 1) for c in oproj_chunks(t, psAux, "po")]),
        2: ([c for t in (12, 13, 14, 15) for c in proj_chunks(t)]
            + [c for t in (2, 3, 4, 5) for c in oproj_chunks(t, psAux, "po")]),
        3: [c for t in (6, 7, 8, 9, 10, 11)
            for c in oproj_chunks(t, psAux, "po")],
    }
    for qc in range(4):
        window(qc, w_fill[qc])
    # tail: last window's o_proj, pipelined over psAux + idle psS slots
    tail = []
    for i, t in enumerate(range(12, 16)):
        tail.append(oproj_chunks(t, psAux if i % 3 == 0 else psS,
                                 "po" if i % 3 == 0 else "st"))
    for group in zip(*tail):  # interleave the 4 tiles' chunks
        for c in group:
            c()
    ctx.close()


_NC_CACHE = None


def _get_nc():
    global _NC_CACHE
    if _NC_CACHE is None:
        _NC_CACHE = _build_nc()
    return _NC_CACHE


def _rope_tables(pos):
    pos = np.asarray(pos, dtype=np.float32)  # [SEQ]
    inv = (1.0 / (np.float32(ROPE_THETA)
                  ** (np.arange(0, HEAD_DIM, 2, dtype=np.float32)
                      / np.float32(HEAD_DIM)))).astype(np.float32)
    fr = pos[:, None] * inv[None, :]                       # [SEQ, 32]
    emb = np.concatenate([fr, fr], axis=-1).astype(np.float32)
    return np.cos(emb).astype(BF16), np.sin(emb).astype(BF16)


def _make_in_maps(input_ids, Wq, Wk, Wv, Wo, position_ids):
    x = np.asarray(input_ids, dtype=np.float32)
    Wq = np.asarray(Wq, dtype=np.float32)
    Wk = np.asarray(Wk, dtype=np.float32)
    Wv = np.asarray(Wv, dtype=np.float32)
    Wo = np.asarray(Wo, dtype=np.float32)
    pos = np.asarray(position_ids)

    maskt = np.triu(np.ones((128, 128), dtype=np.float32)).astype(BF16)

    in_maps = []
    for c in range(N_CORES):
        b, g = c // TP, c % TP
        xTc = np.ascontiguousarray(x[b].T).astype(BF16)
        wq = Wq[:, g * QH * HEAD_DIM:(g + 1) * QH * HEAD_DIM]
        # pair-interleave q head columns: (0,4),(1,5),(2,6),(3,7)
        wq4 = wq.reshape(HIDDEN, QH, HEAD_DIM)
        order = [0, 4, 1, 5, 2, 6, 3, 7]
        wq = wq4[:, order, :].reshape(HIDDEN, QH * HEAD_DIM)
        wk = Wk[:, g * KVH * HEAD_DIM:(g + 1) * KVH * HEAD_DIM]
        wv = Wv[:, g * KVH * HEAD_DIM:(g + 1) * KVH * HEAD_DIM]
        wqkv = np.concatenate([wq, wk, wv], axis=1).astype(BF16)
        wo_s = np.ascontiguousarray(
            Wo[g * F_O:(g + 1) * F_O, :]).astype(BF16)
        cos_t, sin_t = _rope_tables(pos[b])
        in_maps.append({
            "xT": xTc,
            "wqkv": np.ascontiguousarray(wqkv),
            "wo": wo_s,
            "cos": cos_t,
            "sin": sin_t,
            "maskt": maskt,
        })
    return in_maps


def _run(in_maps, trace=False):
    nc = _get_nc()
    kwargs = {}
    if trace:
        _install_profile_hook()
        kwargs["trace"] = True
    return run_bass_kernel_spmd(nc, in_maps, core_ids=list(range(N_CORES)),
                                **kwargs)


def _install_profile_hook():
    """This image's antenv lacks axon_hooks; register the NTFF profile hook
    manually so trace=True yields hardware exec times."""
    if "antenv.axon_hooks" in sys.modules:
        return
    import antenv
    mod = types.ModuleType("antenv.axon_hooks")
    state = {"hook": None}
    mod.set_axon_ntff_profile_hook = lambda h: state.__setitem__("hook", h)
    mod.get_axon_ntff_profile_hook = lambda: state["hook"]
    sys.modules["antenv.axon_hooks"] = mod
    antenv.axon_hooks = mod
    try:
        from trn_agent_boot.trn_boot import _ntff_profile_via_ctypes
        mod.set_axon_ntff_profile_hook(
            _ntff_profile_via_ctypes("/opt/axon/libaxon_pjrt.so"))
    except Exception:
        pass


def kernel(input_ids, Wq, Wk, Wv, Wo, position_ids):
    in_maps = _make_in_maps(input_ids, Wq, Wk, Wv, Wo, position_ids)
    res = _run(in_maps, trace=bool(os.environ.get("KERNEL_TRACE")))
    if os.environ.get("KERNEL_TRACE"):
        print(f"HW exec time: {res.exec_time_ns} ns "
              f"(mean {res.mean_exec_time_ns})")
    # undo the host-side q-head pairing: o_sb feature order is natural
    # h*64+d with h = local head index, same as Wo rows -> nothing to undo.
    out = np.zeros((BATCH, SEQ, HIDDEN), dtype=np.float32)
    for c in range(N_CORES):
        out[c // TP] += res.results[c]["out"].astype(np.float32)
    return out

